# revision 1
# baseline (speedup 1.0000x reference)
"""KV-cached multi-head attention on 8 Trainium2 NeuronCores.

Sharding: 4-way batch (data parallel) x 2-way heads (tensor parallel).
Core c handles batch b = c//2 and head-half h2 = c%2 (8 of 16 heads).
Each core: Q/K/V projections (column-sharded), 8-head causal attention
against the concatenated KV cache, and a row-sharded out-projection
partial. The two partials per batch are summed on the host (+ bo).

Device kernel layout choices:
  - Projections computed in f32r (tf32-class, 1 cyc/row on PE).
  - Q^T/K^T produced head-major [head_dim, tokens]; scores computed
    TRANSPOSED (S^T = K^T.T @ Q^T per 128-key tile) so no P transpose
    is ever needed; exp on ACT (no max subtraction: |scores| <= ~8 for
    this distribution, fp32 exp is safe); softmax denominator via a
    ones-column matmul on PE; PV accumulates O^T = V.T @ P^T directly.
  - P / V / Q^T / K^T in bf16 (PE 1 cyc/row, fp32 PSUM accumulation).
  - Causal mask handled structurally: per 512-query chunk only the
    needed key tiles are computed; the 4 diagonal key tiles use a
    reduced query range plus one 128x128 triangular bf16 multiply.
"""

import sys

sys.path.insert(0, "/opt/trn_rl_repo")

import numpy as np
import ml_dtypes

import concourse.bass as bass  # noqa: F401  (registers AP types)
import concourse.mybir as mybir
import concourse.tile as tile
from concourse import bacc
from concourse.bass_utils import run_bass_kernel_spmd

F32 = mybir.dt.float32
F32R = mybir.dt.float32r
BF16 = mybir.dt.bfloat16
BF = ml_dtypes.bfloat16

D = 2048          # model dim
SQ = 1024         # new tokens per batch
SC = 1024         # cached tokens
SKV = SC + SQ     # total keys
HD = 128          # head dim
HLOC = 8          # heads per core
DH = HLOC * HD    # per-core projected dim (1024)
KC = 17           # contraction chunks (2048 + bias row, padded to 17*128)
KAUG = KC * 128   # 2176
NCORES = 8

EXP = mybir.ActivationFunctionType.Exp


def _emit(tc, nc, prm):
    P = 128

    xq_r = prm["xq"].rearrange("(t p) n -> p t n", p=P)
    xk_r = prm["xk"].rearrange("(t p) n -> p t n", p=P)

    with tc.tile_pool(name="res", bufs=1) as res:
        qt = [res.tile([P, SQ], BF16, name=f"qt{h}", tag=f"qt{h}") for h in range(HLOC)]
        kt = [res.tile([P, SKV], BF16, name=f"kt{h}", tag=f"kt{h}") for h in range(HLOC)]
        vv = [res.tile([P, DH], BF16, name=f"vv{t}", tag=f"vv{t}") for t in range(16)]
        tri = res.tile([P, P], BF16, name="tri", tag="tri")
        ones = res.tile([P, 1], BF16, name="ones", tag="ones")

        nc.sync.dma_start(tri[:], prm["tri"][:])
        nc.vector.memset(ones[:], 1.0)
        # KV cache loads (already bf16, pre-transposed/sliced on host)
        for h in range(HLOC):
            nc.sync.dma_start(kt[h][:, 0:SC], prm["ckt"][P * h : P * (h + 1), :])
        for t in range(8):
            nc.sync.dma_start(vv[t][:], prm["cv"][P * t : P * (t + 1), :])

        # ---------------- projections ----------------
        with (
            tc.tile_pool(name="pjx", bufs=2) as pjx,
            tc.tile_pool(name="pjw", bufs=4) as pjw,
            tc.tile_pool(name="pjps", bufs=1, space="PSUM") as pjps,
        ):
            # V: out[tok_tile, dout] = xv.T @ wv   (natural [tok, dh] layout)
            for cd in range(2):
                ps_t = [
                    pjps.tile([P, 512], F32, name=f"vps{cd}_{m}", tag=f"ps{m}")
                    for m in range(8)
                ]
                for k in range(KC):
                    xv_t = pjw.tile([P, SQ], F32R, name=f"xv{cd}_{k}", tag="xvk")
                    nc.sync.dma_start(xv_t[:], prm["xv"][P * k : P * (k + 1), :])
                    wv_t = pjw.tile([P, 512], F32R, name=f"wv{cd}_{k}", tag="wvk")
                    nc.sync.dma_start(
                        wv_t[:], prm["wv"][P * k : P * (k + 1), 512 * cd : 512 * (cd + 1)]
                    )
                    for m in range(8):
                        nc.tensor.matmul(
                            ps_t[m][:],
                            xv_t[:, P * m : P * (m + 1)],
                            wv_t[:],
                            start=(k == 0),
                            stop=(k == KC - 1),
                        )
                for m in range(8):
                    nc.scalar.copy(vv[8 + m][:, 512 * cd : 512 * (cd + 1)], ps_t[m][:])

            # K then Q: out[dout_tile, tok] = w.T @ x  (transposed layout)
            for name_x, xr, name_w, dest, col0 in (
                ("xk", xk_r, "wk", kt, SC),
                ("xq", xq_r, "wq", qt, 0),
            ):
                for c in range(2):
                    xc = pjx.tile([P, KC, 512], F32R, name=f"{name_x}c{c}", tag="pjx")
                    nc.sync.dma_start(xc[:], xr[:, :, 512 * c : 512 * (c + 1)])
                    ps_m = [
                        pjps.tile([P, 512], F32, name=f"{name_w}ps{c}_{m}", tag=f"ps{m}")
                        for m in range(8)
                    ]
                    for k in range(KC):
                        w_t = pjw.tile([P, DH], F32R, name=f"{name_w}{c}_{k}", tag="wk")
                        nc.sync.dma_start(w_t[:], prm[name_w][P * k : P * (k + 1), :])
                        for m in range(8):
                            nc.tensor.matmul(
                                ps_m[m][:],
                                w_t[:, P * m : P * (m + 1)],
                                xc[:, k, :],
                                start=(k == 0),
                                stop=(k == KC - 1),
                            )
                    for m in range(8):
                        nc.scalar.copy(
                            dest[m][:, col0 + 512 * c : col0 + 512 * c + 512], ps_m[m][:]
                        )

        # ---------------- attention ----------------
        with tc.tile_pool(name="at_p", bufs=1) as at_p:
          at = [
              at_p.tile([P, SQ], F32R, name=f"at{t}", tag=f"at{t}") for t in range(HLOC)
          ]
          with (
            tc.tile_pool(name="stps", bufs=4, space="PSUM") as stps,
            tc.tile_pool(name="ops", bufs=2, space="PSUM") as ops,
            tc.tile_pool(name="dps", bufs=2, space="PSUM") as dps,
            tc.tile_pool(name="ptp", bufs=8) as ptp,
            tc.tile_pool(name="bcp", bufs=3) as bcp,
          ):
            for h in range(HLOC):
                for c in range(2):
                    n_full = 8 + 4 * c
                    n_kv = n_full + 4
                    q_sl = slice(512 * c, 512 * (c + 1))
                    o_ps = ops.tile([P, 512], F32, name=f"o{h}_{c}", tag="o")
                    d_ps = dps.tile([1, 512], F32, name=f"d{h}_{c}", tag="d")
                    for g in range(n_kv):
                        j = g - n_full  # >= 0 on diagonal tiles
                        st = stps.tile([P, 512], F32, name=f"st{h}_{c}_{g}", tag="st")
                        pt = ptp.tile([P, 512], BF16, name=f"pt{h}_{c}_{g}", tag="pt")
                        if j < 0:
                            nc.tensor.matmul(
                                st[:], kt[h][:, P * g : P * (g + 1)], qt[h][:, q_sl],
                                start=True, stop=True,
                            )
                            nc.scalar.activation(pt[:], st[:], EXP)
                        else:
                            o0 = 128 * j
                            nc.tensor.matmul(
                                st[:, o0:512],
                                kt[h][:, P * g : P * (g + 1)],
                                qt[h][:, 512 * c + o0 : 512 * (c + 1)],
                                start=True, stop=True,
                            )
                            if o0:
                                nc.vector.memset(pt[:, 0:o0], 0.0)
                            nc.scalar.activation(pt[:, o0:512], st[:, o0:512], EXP)
                            nc.vector.tensor_mul(
                                pt[:, o0 : o0 + P], pt[:, o0 : o0 + P], tri[:]
                            )
                        nc.tensor.matmul(
                            o_ps[:], vv[g][:, P * h : P * (h + 1)], pt[:],
                            start=(g == 0), stop=(g == n_kv - 1),
                        )
                        nc.tensor.matmul(
                            d_ps[:], ones[:], pt[:],
                            start=(g == 0), stop=(g == n_kv - 1),
                        )
                    rec = bcp.tile([1, 512], F32, name=f"rec{h}_{c}", tag="rec")
                    nc.vector.reciprocal(rec[:], d_ps[:])
                    bc = bcp.tile([P, 512], F32, name=f"bc{h}_{c}", tag="bc")
                    nc.gpsimd.partition_broadcast(bc[:], rec[:])
                    nc.vector.tensor_mul(at[h][:, q_sl], o_ps[:], bc[:])

          # ---------------- out-projection ----------------
          with (
              tc.tile_pool(name="wop", bufs=3) as wop,
              tc.tile_pool(name="out_ps", bufs=4, space="PSUM") as out_ps,
              tc.tile_pool(name="outs", bufs=4) as outs,
          ):
              for m in range(16):
                  wo_t = wop.tile([P, 8, P], F32R, name=f"wo{m}", tag="wo")
                  nc.sync.dma_start(
                      wo_t[:], prm["wo"][m].rearrange("(t p) n -> p t n", p=P)
                  )
                  for c in range(2):
                      op = out_ps.tile([P, 512], F32, name=f"op{m}_{c}", tag="op")
                      for t in range(HLOC):
                          nc.tensor.matmul(
                              op[:], wo_t[:, t, :], at[t][:, 512 * c : 512 * (c + 1)],
                              start=(t == 0), stop=(t == HLOC - 1),
                          )
                      ob = outs.tile([P, 512], F32, name=f"ob{m}_{c}", tag="ob")
                      nc.scalar.copy(ob[:], op[:])
                      nc.sync.dma_start(
                          prm["outT"][P * m : P * (m + 1), 512 * c : 512 * (c + 1)],
                          ob[:],
                      )


def build():
    nc = bacc.Bacc(None, target_bir_lowering=False)
    prm = {}
    for n, shape, dt in (
        ("xq", [KAUG, SQ], F32R),
        ("xk", [KAUG, SQ], F32R),
        ("xv", [KAUG, SQ], F32R),
        ("wq", [KAUG, DH], F32R),
        ("wk", [KAUG, DH], F32R),
        ("wv", [KAUG, DH], F32R),
        ("wo", [16, DH, 128], F32R),
        ("ckt", [DH, SC], BF16),
        ("cv", [SC, DH], BF16),
        ("tri", [128, 128], BF16),
    ):
        prm[n] = nc.declare_dram_parameter(n, shape, dt, isOutput=False)
    prm["outT"] = nc.declare_dram_parameter("outT", [D, SQ], F32, isOutput=True)
    with tile.TileContext(nc) as tc:
        _emit(tc, nc, prm)
    nc.compile()
    return nc


def make_in_maps(query, key, value, cached_k, cached_v, Wq, bq, Wk, bk, Wv, bv, Wo, bo):
    """Per-core host prep: slice + transpose + bias-augment + casts."""
    s = float(np.sqrt(HD))
    tri = np.triu(np.ones((128, 128), dtype=np.float32)).astype(BF)

    def aug_x(x):  # [SQ, D] -> [KAUG, SQ] with ones row at 2048
        a = np.zeros((KAUG, SQ), dtype=np.float32)
        a[:D] = np.ascontiguousarray(x.T)
        a[D] = 1.0
        return a

    def aug_w(w, b):  # w [DH, D] (rows = out features), b [DH] -> [KAUG, DH]
        a = np.zeros((KAUG, DH), dtype=np.float32)
        a[:D] = np.ascontiguousarray(w.T)
        a[D] = b
        return a

    in_maps = []
    for c in range(NCORES):
        b, h2 = c // 2, c % 2
        hs = slice(DH * h2, DH * (h2 + 1))
        wo_s = np.ascontiguousarray(Wo[:, hs].T)  # [DH, D]
        in_maps.append(
            {
                "xq": aug_x(query[b]),
                "xk": aug_x(key[b]),
                "xv": aug_x(value[b]),
                "wq": aug_w(Wq[hs] / s, bq[hs] / s),
                "wk": aug_w(Wk[hs], bk[hs]),
                "wv": aug_w(Wv[hs], bv[hs]),
                "wo": np.ascontiguousarray(
                    wo_s.reshape(DH, 16, 128).transpose(1, 0, 2)
                ),
                "ckt": np.ascontiguousarray(cached_k[b][:, hs].T).astype(BF),
                "cv": np.ascontiguousarray(cached_v[b][:, hs]).astype(BF),
                "tri": tri,
            }
        )
    return in_maps


_NC_CACHE = []


def get_nc():
    if not _NC_CACHE:
        _NC_CACHE.append(build())
    return _NC_CACHE[0]


def assemble(results, bo):
    out = np.empty((4, SQ, D), dtype=np.float32)
    for b in range(4):
        acc = results[2 * b]["outT"] + results[2 * b + 1]["outT"]  # [D, SQ]
        out[b] = acc.T + bo[None, :]
    return out


def kernel(query, key, value, cached_k, cached_v, Wq, bq, Wk, bk, Wv, bv, Wo, bo):
    query = np.asarray(query, dtype=np.float32)
    key = np.asarray(key, dtype=np.float32)
    value = np.asarray(value, dtype=np.float32)
    cached_k = np.asarray(cached_k, dtype=np.float32)
    cached_v = np.asarray(cached_v, dtype=np.float32)
    Wq, bq = np.asarray(Wq, np.float32), np.asarray(bq, np.float32)
    Wk, bk = np.asarray(Wk, np.float32), np.asarray(bk, np.float32)
    Wv, bv = np.asarray(Wv, np.float32), np.asarray(bv, np.float32)
    Wo, bo = np.asarray(Wo, np.float32), np.asarray(bo, np.float32)

    nc = get_nc()
    in_maps = make_in_maps(
        query, key, value, cached_k, cached_v, Wq, bq, Wk, bk, Wv, bv, Wo, bo
    )
    res = run_bass_kernel_spmd(nc, in_maps, list(range(NCORES)))
    return assemble(res.results, bo)



# revision 75
# speedup vs baseline: 1.6045x; 1.6045x over previous
"""KV-cached multi-head attention on 8 Trainium2 NeuronCores.

Sharding: 4-way batch (data parallel) x 2-way heads (tensor parallel).
Core c handles batch b = c//2 and head-half h2 = c%2 (8 of 16 heads).
Each core: Q/K/V projections (column-sharded), 8-head causal attention
against the concatenated KV cache, and a row-sharded out-projection
partial. The two partials per batch are summed on the host (+ bo).

All matmul streams are bf16 (1 col/cycle on PE); the schedule keeps the
PE at its causal-minimum cycle count (~304us) with <5% idle:
  - One shared 8-bank PSUM pool, retagged per phase (no pool-close
    drains): projections slice 8 accumulators out of {d0,d1,o0,o1,
    x0,x1}; attention rotates [P,1024] score PAIRS on d0/d1, O^T on
    o0/o1; out-projection rotates x0/x1 (hoistable by the scheduler
    into the ACT-bound attention window).
  - Paired exps: two score tiles share one 2-bank PSUM tile and one
    [P,1024] ACT exp lands in two adjacent ptb slots, halving the
    per-instruction ACT overhead that saturates attention.
  - Softmax denominator off the PE: bf16 pairwise TT-add tree on DVE
    (2x mode) + in-place causally-restricted diagonal adds, GPSIMD
    partition_all_reduce, DVE reciprocal/multiply deferred one chunk.
  - Biases: K/Q per-partition adds ride the PSUM drain (ACT/DVE split,
    interleaved with the last k-batch); V-bias folded to the host
    (cv - bv, bo + bv @ Wo.T) since softmax weights sum to 1.
  - Q chunk-1 projection and the out-projection weight prefetch are
    deferred into the attention-c0 window to feed the PE while ACT
    churns exps; attention PVs flow through a cross-chunk software
    pipeline so chunk boundaries never drain the PE.
  - DMA: 4-k-tile batched loads amortize the ~0.7us per-DMA fixed
    cost; KV cache trickles in on the Pool (SWDGE) queue; a 1-column
    warmup matmul at t~0 locks the PE p-state ramp before real work.
"""

import sys

sys.path.insert(0, "/opt/trn_rl_repo")

import numpy as np
import ml_dtypes

import concourse.bass as bass  # noqa: F401  (registers AP types)
import concourse.mybir as mybir
import concourse.tile as tile
from concourse import bacc
from concourse import bass_isa
from concourse.bass_utils import run_bass_kernel_spmd

F32 = mybir.dt.float32
BF16 = mybir.dt.bfloat16
BF = ml_dtypes.bfloat16

D = 2048          # model dim
SQ = 1024         # new tokens per batch
SC = 1024         # cached tokens
SKV = SC + SQ     # total keys
HD = 128          # head dim
HLOC = 8          # heads per core
DH = HLOC * HD    # per-core projected dim (1024)
KT = 16           # contraction tiles (2048 / 128)
NCORES = 8
P = 128

EXP = mybir.ActivationFunctionType.Exp


def _emit(tc, nc, prm):
    with (
        tc.tile_pool(name="res", bufs=1) as res,
        tc.tile_pool(name="wres", bufs=1) as wres,
        tc.tile_pool(name="xs", bufs=4) as xs,
        tc.tile_pool(name="ptp", bufs=2) as ptp,
        tc.tile_pool(name="trp", bufs=2) as trp,
        tc.tile_pool(name="accp", bufs=2) as accp,
        tc.tile_pool(name="dnp", bufs=2) as dnp,
        tc.tile_pool(name="outs", bufs=4) as outs,
        tc.tile_pool(name="ps8", bufs=1, space="PSUM") as ps8,
    ):
        qt = [res.tile([P, SQ], BF16, name=f"qt{h}", tag=f"qt{h}") for h in range(HLOC)]
        kt = [res.tile([P, SKV], BF16, name=f"kt{h}", tag=f"kt{h}") for h in range(HLOC)]
        vv = [res.tile([P, DH], BF16, name=f"vv{t}", tag=f"vv{t}") for t in range(16)]
        at = [res.tile([P, SQ], BF16, name=f"at{h}", tag=f"at{h}") for h in range(HLOC)]
        tri = res.tile([P, P], BF16, name="tri", tag="tri")
        bq2 = res.tile([P, 8], F32, name="bq2", tag="bq2")
        bk2 = res.tile([P, 8], F32, name="bk2", tag="bk2")

        # prologue on the Pool (SWDGE) queue: constants first (needed by
        # the projection phases), then KV-cache loads that trickle in
        # during projections without occupying the SP queue.
        warm = ps8.tile([P, 512], F32, name="warm", tag="x1")
        nc.tensor.matmul(warm[0:1, 0:1], tri[:, 0:1], tri[:, 0:1], start=True, stop=True)
        # first V-phase weight tile via SWDGE so it lands in parallel with xv
        wv00 = xs.tile([P, 512], BF16, name="wv0_0", tag="xs0")
        nc.gpsimd.dma_start(wv00[:], prm["wv"][0:P, 0:512])

        # ---------------- projections: V, then K, then Q ----------------
        # Resident operands live in four [P, 4, *] batch tiles (one DMA
        # per 4 k-tiles amortizes the per-DMA fixed cost); streams are
        # [P, 4, 512] batches in a 3-deep ring. k-tile k = (g=k//4, t=k%4).
        def bload(tile_, name, g, cols):
            nc.sync.dma_start(
                tile_[:],
                prm[name][512 * g : 512 * (g + 1), cols].rearrange("(t p) n -> p t n", p=P),
            )

        # V: natural layout [tok, feat]; x resident, w streamed.
        xvb = [wres.tile([P, 4, SQ], BF16, name=f"xv{g}", tag=f"big{g}") for g in range(4)]
        wvbs = {}
        for cd in range(2):
            for g in range(4):
                wvbs[(cd, g)] = xs.tile([P, 4, 512], BF16, name=f"wv{cd}_{g}", tag="xs")
        # startup: smallest pieces first so the k=0 matmuls begin ASAP
        nc.sync.dma_start(xvb[0][:, 0, 0:512], prm["xv"][0:P, 0:512])
        nc.sync.dma_start(xvb[0][:, 0, 512:SQ], prm["xv"][0:P, 512:SQ])
        nc.scalar.dma_start(
            wvbs[(0, 0)][:, 1:4, :],
            prm["wv"][P:512, 0:512].rearrange("(t p) n -> p t n", p=P),
        )
        nc.sync.dma_start(xvb[0][:, 1, :], prm["xv"][P : 2 * P, :])
        nc.sync.dma_start(
            xvb[0][:, 2:4, :],
            prm["xv"][2 * P : 512, :].rearrange("(t p) n -> p t n", p=P),
        )
        nc.sync.dma_start(
            xvb[1][:, 0:2, :],
            prm["xv"][512:768, :].rearrange("(t p) n -> p t n", p=P),
        )
        nc.sync.dma_start(
            xvb[1][:, 2:4, :],
            prm["xv"][768:1024, :].rearrange("(t p) n -> p t n", p=P),
        )
        bload(wvbs[(0, 1)], "wv", 1, slice(0, 512))
        for g in range(2, 4):
            bload(xvb[g], "xv", g, slice(0, SQ))
            bload(wvbs[(0, g)], "wv", g, slice(0, 512))
        # cd1 weight stream + constants + KV cache trickle in on the Pool
        # queue; SP is left free for the K-phase prefetch during cd1.
        for g in range(4):
            bload(wvbs[(1, g)], "wv", g, slice(512, 1024))
        nc.gpsimd.dma_start(tri[:], prm["tri"][:])
        nc.gpsimd.dma_start(bq2[:], prm["bq2"][:])
        nc.gpsimd.dma_start(bk2[:], prm["bk2"][:])
        for h in range(HLOC):
            nc.gpsimd.dma_start(kt[h][:, 0:SC], prm["ckt"][P * h : P * (h + 1), :])
        for t in range(8):
            nc.gpsimd.dma_start(vv[t][:], prm["cv"][P * t : P * (t + 1), :])

        def proj_ps(pref):
            d0 = ps8.tile([P, 1024], F32, name=f"{pref}d0", tag="d0")
            d1 = ps8.tile([P, 1024], F32, name=f"{pref}d1", tag="d1")
            return [
                d0[:, 0:512], d0[:, 512:1024], d1[:, 0:512], d1[:, 512:1024],
                ps8.tile([P, 512], F32, name=f"{pref}o0", tag="o0")[:],
                ps8.tile([P, 512], F32, name=f"{pref}o1", tag="o1")[:],
                ps8.tile([P, 512], F32, name=f"{pref}x0", tag="x0")[:],
                ps8.tile([P, 512], F32, name=f"{pref}x1", tag="x1")[:],
            ]

        for cd in range(2):
            ps = proj_ps(f"vps{cd}_")

            for g in range(4):
                wvb = wvbs[(cd, g)]
                for t in range(4):
                    k = 4 * g + t
                    rhs = wv00[:] if (cd == 0 and k == 0) else wvb[:, t, :]
                    for m in range(8):
                        nc.tensor.matmul(
                            ps[m], xvb[g][:, t, P * m : P * (m + 1)], rhs,
                            start=(k == 0), stop=(k == KT - 1),
                        )
                        if k == KT - 1:
                            # drain the PSUM->SBUF copy while later m's finish
                            dsl = vv[8 + m][:, 512 * cd : 512 * (cd + 1)]
                            if m % 2 == 0:
                                nc.scalar.copy(dsl, ps[m])
                            else:
                                nc.vector.tensor_copy(dsl, ps[m])

        # K then Q: transposed layout [feat, tok]; w resident, x streamed.
        # Q chunk 1 is deferred into the attention-c0 window (ACT-bound
        # there, so its matmuls fill otherwise-idle PE time).
        wtq = None
        for wname, xname, dest, col0, bias2, cs in (
            ("wk", "xk", kt, SC, bk2, (0, 1)),
            ("wq", "xq", qt, 0, bq2, (0,)),
        ):
            wt = [wres.tile([P, 4, DH], BF16, name=f"{wname}{g}", tag=f"big{g}") for g in range(4)]
            if wname == "wq":
                wtq = wt
            for c in cs:
                ps = proj_ps(f"{wname}ps{c}_")
                for g in range(4):
                    if c == 0:
                        bload(wt[g], wname, g, slice(0, DH))
                    xb = xs.tile([P, 4, 512], BF16, name=f"{xname}{c}_{g}", tag="xs")
                    bload(xb, xname, g, slice(512 * c, 512 * (c + 1)))
                    for t in range(4):
                        k = 4 * g + t
                        for m in range(8):
                            nc.tensor.matmul(
                                ps[m], wt[g][:, t, P * m : P * (m + 1)], xb[:, t, :],
                                start=(k == 0), stop=(k == KT - 1),
                            )
                            if k == KT - 1:
                                dsl = dest[m][:, col0 + 512 * c : col0 + 512 * c + 512]
                                if m % 2 == 0:
                                    nc.scalar.add(dsl, ps[m], bias2[:, m : m + 1])
                                else:
                                    nc.vector.tensor_scalar_add(
                                        dsl, ps[m], bias2[:, m : m + 1]
                                    )

        # ---------------- attention ----------------
        def qc1_pair(p):
            # deferred Q projection for chunk 1, two feature tiles at a time
            # (PSUM banks x0/x1 are free until out-projection starts)
            ps2 = [
                ps8.tile([P, 512], F32, name=f"qps1_{p}_{i}", tag=f"x{i}")
                for i in range(2)
            ]
            for g in range(4):
                xb = xs.tile([P, 4, 512], BF16, name=f"xq1_{p}_{g}", tag="xs")
                bload(xb, "xq", g, slice(512, 1024))
                for t in range(4):
                    k = 4 * g + t
                    for i in range(2):
                        m = 2 * p + i
                        nc.tensor.matmul(
                            ps2[i][:], wtq[g][:, t, P * m : P * (m + 1)], xb[:, t, :],
                            start=(k == 0), stop=(k == KT - 1),
                        )
                        if k == KT - 1:
                            if i == 0:
                                nc.scalar.add(
                                    qt[m][:, 512:1024], ps2[i][:], bq2[:, m : m + 1]
                                )
                            else:
                                nc.vector.tensor_scalar_add(
                                    qt[m][:, 512:1024], ps2[i][:], bq2[:, m : m + 1]
                                )


        wob = [wres.tile([P, 4, DH], BF16, name=f"wo{i}", tag=f"big{i}") for i in range(4)]

        def wo_prefetch():
            # wob[2*jh + dh][:, tt, :] = Wo^T rows 512*jh + 128*tt, cols DH*dh ..
            for jh in range(2):
                for dh in range(2):
                    nc.sync.dma_start(
                        wob[2 * jh + dh][:],
                        prm["wo"][
                            512 * jh : 512 * (jh + 1), DH * dh : DH * (dh + 1)
                        ].rearrange("(t p) n -> p t n", p=P),
                    )

        pending = []  # deferred (o_ps, d_all, h, c) normalizes
        pvq = []  # cross-chunk PV pipeline (emitted PDP pairs behind scores)
        PDP = 1

        def flush_normalize():
            o_ps, d_all, h, c = pending.pop(0)
            rec = dnp.tile([P, 512], F32, name=f"rec{h}_{c}", tag="rec")
            nc.vector.reciprocal(rec[:], d_all[:])
            nc.vector.tensor_mul(at[h][:, 512 * c : 512 * (c + 1)], o_ps[:], rec[:])

        def attn(h, c, idx):
            n_full = 8 + 4 * c
            n_kv = n_full + 4
            q_sl = slice(512 * c, 512 * (c + 1))
            ptb = ptp.tile([P, 16, 512], BF16, name=f"ptb{h}_{c}", tag="ptb")
            ts = trp.tile([P, 6, 512], BF16, name=f"ts{h}_{c}", tag="ts")
            acc = accp.tile([P, 512], BF16, name=f"acc{h}_{c}", tag="acc")
            d_all = dnp.tile([P, 512], F32, name=f"d{h}_{c}", tag="d")
            o_ps = ps8.tile([P, 512], F32, name=f"o{h}_{c}", tag=f"o{idx % 2}")

            # bf16 pairwise-tree denominator over the full (non-diagonal)
            # tiles; all operands 2-byte so DVE runs in 2x mode. Pair adds
            # are emitted as exps complete; the tree root lands in acc and
            # the diagonal tiles are then added in place, each starting at
            # its own query block so causality is preserved.
            def tree_root():
                if n_full == 8:
                    nc.vector.tensor_add(ts[:, 4, :], ts[:, 0, :], ts[:, 1, :])
                    nc.vector.tensor_add(ts[:, 5, :], ts[:, 2, :], ts[:, 3, :])
                    nc.vector.tensor_add(acc[:], ts[:, 4, :], ts[:, 5, :])
                else:  # 12 full tiles -> 6 pairs
                    nc.vector.tensor_add(ts[:, 0, :], ts[:, 0, :], ts[:, 1, :])
                    nc.vector.tensor_add(ts[:, 2, :], ts[:, 2, :], ts[:, 3, :])
                    nc.vector.tensor_add(ts[:, 4, :], ts[:, 4, :], ts[:, 5, :])
                    nc.vector.tensor_add(ts[:, 0, :], ts[:, 0, :], ts[:, 2, :])
                    nc.vector.tensor_add(acc[:], ts[:, 0, :], ts[:, 4, :])

            def scores_pair(u):
                # two score tiles per double-bank PSUM tile; full pairs get
                # one [P,1024] exp straight into adjacent ptb slots
                dp = ps8.tile([P, 1024], F32, name=f"st{h}_{c}_{u}", tag=f"d{u % 2}")
                for half in range(2):
                    g = 2 * u + half
                    j = g - n_full
                    base = 512 * half
                    if j < 0:
                        nc.tensor.matmul(
                            dp[:, base : base + 512],
                            kt[h][:, P * g : P * (g + 1)], qt[h][:, q_sl],
                            start=True, stop=True,
                        )
                    else:
                        o0 = P * j
                        nc.tensor.matmul(
                            dp[:, base + o0 : base + 512],
                            kt[h][:, P * g : P * (g + 1)],
                            qt[h][:, 512 * c + o0 : 512 * (c + 1)],
                            start=True, stop=True,
                        )
                g = 2 * u + 1
                if g < n_full:
                    nc.scalar.activation(ptb[:, 2 * u : 2 * u + 2, :], dp[:], EXP)
                    nc.vector.tensor_add(
                        ts[:, u, :], ptb[:, g - 1, :], ptb[:, g, :]
                    )
                    if g == n_full - 1:
                        tree_root()
                else:
                    first_diag = 2 * u == n_full
                    if first_diag:
                        # j=0 spans the full half; one exp covers both tiles
                        # (the 128 garbage cols between are never read)
                        nc.scalar.activation(ptb[:, 2 * u : 2 * u + 2, :], dp[:], EXP)
                    for half in range(2):
                        g = 2 * u + half
                        o0 = P * (g - n_full)
                        base = 512 * half
                        if not first_diag:
                            nc.scalar.activation(
                                ptb[:, g, o0:512], dp[:, base + o0 : base + 512], EXP
                            )
                        nc.vector.tensor_mul(
                            ptb[:, g, o0 : o0 + P], ptb[:, g, o0 : o0 + P], tri[:]
                        )
                        nc.vector.tensor_add(
                            acc[:, o0:512], acc[:, o0:512], ptb[:, g, o0:512]
                        )

            def pv(g):
                j = g - n_full
                if j < 0:
                    nc.tensor.matmul(
                        o_ps[:], vv[g][:, P * h : P * (h + 1)], ptb[:, g, :],
                        start=(g == 0), stop=False,
                    )
                else:
                    o0 = P * j
                    # retiring query block: last writer of cols [o0, o0+128)
                    nc.tensor.matmul(
                        o_ps[:, o0 : o0 + P], vv[g][:, P * h : P * (h + 1)],
                        ptb[:, g, o0 : o0 + P], start=False, stop=True,
                    )
                    if o0 + P < 512:
                        nc.tensor.matmul(
                            o_ps[:, o0 + P : 512], vv[g][:, P * h : P * (h + 1)],
                            ptb[:, g, o0 + P : 512], start=False, stop=(j == 3),
                        )

            def finish():
                nc.gpsimd.partition_all_reduce(
                    d_all[:], acc[:], channels=P, reduce_op=bass_isa.ReduceOp.add
                )
                pending.append((o_ps, d_all, h, c))
                if len(pending) >= 2:
                    flush_normalize()

            # feed the global cross-chunk pipeline: one entry per pair;
            # the last pair carries the chunk finisher (pall + normalize)
            n_pairs = n_kv // 2
            for u in range(n_pairs):
                scores_pair(u)

                def mk(g0=2 * u, fin=(finish if u == n_pairs - 1 else None)):
                    def emit():
                        pv(g0)
                        pv(g0 + 1)
                        if fin is not None:
                            fin()
                    return emit

                pvq.append(mk())
                while len(pvq) > PDP:
                    pvq.pop(0)()

        def outproj(c, ms=range(16), split_last=False):
            for m in ms:
                op = ps8.tile([P, 512], F32, name=f"op{m}_{c}", tag=f"x{m % 2}")
                for t in range(HLOC):
                    nc.tensor.matmul(
                        op[:],
                        wob[2 * (t // 4) + m // 8][:, t % 4, P * (m % 8) : P * (m % 8 + 1)],
                        at[t][:, 512 * c : 512 * (c + 1)],
                        start=(t == 0), stop=(t == HLOC - 1),
                    )
                ob = outs.tile([P, 512], BF16, name=f"ob{m}_{c}", tag="ob")
                if split_last and m == 15:
                    # two engines copy halves in parallel to shorten the tail
                    nc.scalar.copy(ob[:, 0:256], op[:, 0:256])
                    nc.vector.tensor_copy(ob[:, 256:512], op[:, 256:512])
                    nc.sync.dma_start(
                        prm["outT"][P * m : P * (m + 1), 512 * c : 512 * c + 256],
                        ob[:, 0:256],
                    )
                    nc.sync.dma_start(
                        prm["outT"][P * m : P * (m + 1), 512 * c + 256 : 512 * (c + 1)],
                        ob[:, 256:512],
                    )
                else:
                    nc.scalar.copy(ob[:], op[:])
                    nc.sync.dma_start(
                        prm["outT"][P * m : P * (m + 1), 512 * c : 512 * (c + 1)], ob[:]
                    )

        idx = 0
        for h in range(HLOC):
            attn(h, 0, idx)
            idx += 1
            if h in (0, 2, 4):
                qc1_pair(h // 2)
            if h == 6:
                wo_prefetch()
        for h in range(HLOC):
            attn(h, 1, idx)
            idx += 1
            if h == 0:
                qc1_pair(3)
        while pvq:
            pvq.pop(0)()
        while pending:
            flush_normalize()
        outproj(0)
        outproj(1)


def build():
    nc = bacc.Bacc(None, target_bir_lowering=False)
    prm = {}
    for n, shape, dt in (
        ("xq", [D, SQ], BF16),
        ("xk", [D, SQ], BF16),
        ("xv", [D, SQ], BF16),
        ("wq", [D, DH], BF16),
        ("wk", [D, DH], BF16),
        ("wv", [D, DH], BF16),
        ("wo", [DH, D], BF16),
        ("ckt", [DH, SC], BF16),
        ("cv", [SC, DH], BF16),
        ("tri", [P, P], BF16),
        ("bq2", [P, 8], F32),
        ("bk2", [P, 8], F32),
    ):
        prm[n] = nc.declare_dram_parameter(n, shape, dt, isOutput=False)
    prm["outT"] = nc.declare_dram_parameter("outT", [D, SQ], BF16, isOutput=True)
    with tile.TileContext(nc) as tc:
        _emit(tc, nc, prm)
    nc.compile()
    return nc


def make_in_maps(query, key, value, cached_k, cached_v, Wq, bq, Wk, bk, Wv, bv, Wo, bo):
    """Per-core host prep: slice + transpose + casts (bf16 streams)."""
    s = float(np.sqrt(HD))
    tri = np.triu(np.ones((P, P), dtype=np.float32)).astype(BF)

    in_maps = []
    for c in range(NCORES):
        b, h2 = c // 2, c % 2
        hs = slice(DH * h2, DH * (h2 + 1))
        in_maps.append(
            {
                "xq": np.ascontiguousarray(query[b].T).astype(BF),
                "xk": np.ascontiguousarray(key[b].T).astype(BF),
                "xv": np.ascontiguousarray(value[b].T).astype(BF),
                "wq": np.ascontiguousarray(Wq[hs].T / s).astype(BF),
                "wk": np.ascontiguousarray(Wk[hs].T).astype(BF),
                "wv": np.ascontiguousarray(Wv[hs].T).astype(BF),
                "wo": np.ascontiguousarray(Wo[:, hs].T).astype(BF),
                "ckt": np.ascontiguousarray(cached_k[b][:, hs].T).astype(BF),
                "cv": np.ascontiguousarray(cached_v[b][:, hs] - bv[hs]).astype(BF),
                "tri": tri,
                "bq2": np.ascontiguousarray((bq[hs] / s).reshape(8, P).T.astype(np.float32)),
                "bk2": np.ascontiguousarray(bk[hs].reshape(8, P).T.astype(np.float32)),
            }
        )
    return in_maps


_NC_CACHE = []


def get_nc():
    if not _NC_CACHE:
        _NC_CACHE.append(build())
    return _NC_CACHE[0]


def assemble(results, bo):
    out = np.empty((4, SQ, D), dtype=np.float32)
    for b in range(4):
        acc = results[2 * b]["outT"].astype(np.float32) + results[
            2 * b + 1
        ]["outT"].astype(np.float32)  # [D, SQ]
        out[b] = acc.T + bo[None, :]
    return out


def kernel(query, key, value, cached_k, cached_v, Wq, bq, Wk, bk, Wv, bv, Wo, bo):
    query = np.asarray(query, dtype=np.float32)
    key = np.asarray(key, dtype=np.float32)
    value = np.asarray(value, dtype=np.float32)
    cached_k = np.asarray(cached_k, dtype=np.float32)
    cached_v = np.asarray(cached_v, dtype=np.float32)
    Wq, bq = np.asarray(Wq, np.float32), np.asarray(bq, np.float32)
    Wk, bk = np.asarray(Wk, np.float32), np.asarray(bk, np.float32)
    Wv, bv = np.asarray(Wv, np.float32), np.asarray(bv, np.float32)
    Wo, bo = np.asarray(Wo, np.float32), np.asarray(bo, np.float32)

    nc = get_nc()
    in_maps = make_in_maps(
        query, key, value, cached_k, cached_v, Wq, bq, Wk, bk, Wv, bv, Wo, bo
    )
    res = run_bass_kernel_spmd(nc, in_maps, list(range(NCORES)))
    return assemble(res.results, bo + bv @ Wo.T)


# revision 76
# speedup vs baseline: 1.6047x; 1.0001x over previous
"""KV-cached multi-head attention on 8 Trainium2 NeuronCores.

Sharding: 4-way batch (data parallel) x 2-way heads (tensor parallel).
Core c handles batch b = c//2 and head-half h2 = c%2 (8 of 16 heads).
Each core: Q/K/V projections (column-sharded), 8-head causal attention
against the concatenated KV cache, and a row-sharded out-projection
partial. The two partials per batch are summed on the host (+ bo).

All matmul streams are bf16 (1 col/cycle on PE); the schedule keeps the
PE at its causal-minimum cycle count (~304us) with <5% idle:
  - One shared 8-bank PSUM pool, retagged per phase (no pool-close
    drains): projections slice 8 accumulators out of {d0,d1,o0,o1,
    x0,x1}; attention rotates [P,1024] score PAIRS on d0/d1, O^T on
    o0/o1; out-projection rotates x0/x1 (hoistable by the scheduler
    into the ACT-bound attention window).
  - Paired exps: two score tiles share one 2-bank PSUM tile and one
    [P,1024] ACT exp lands in two adjacent ptb slots, halving the
    per-instruction ACT overhead that saturates attention.
  - Softmax denominator off the PE: bf16 pairwise TT-add tree on DVE
    (2x mode) + in-place causally-restricted diagonal adds, GPSIMD
    partition_all_reduce, DVE reciprocal/multiply deferred one chunk.
  - Biases: K/Q per-partition adds ride the PSUM drain (ACT/DVE split,
    interleaved with the last k-batch); V-bias folded to the host
    (cv - bv, bo + bv @ Wo.T) since softmax weights sum to 1.
  - Q chunk-1 projection and the out-projection weight prefetch are
    deferred into the attention-c0 window to feed the PE while ACT
    churns exps; attention PVs flow through a cross-chunk software
    pipeline so chunk boundaries never drain the PE.
  - DMA: 4-k-tile batched loads amortize the ~0.7us per-DMA fixed
    cost; KV cache trickles in on the Pool (SWDGE) queue; a 1-column
    warmup matmul at t~0 locks the PE p-state ramp before real work.
"""

import sys

sys.path.insert(0, "/opt/trn_rl_repo")

import numpy as np
import ml_dtypes

import concourse.bass as bass  # noqa: F401  (registers AP types)
import concourse.mybir as mybir
import concourse.tile as tile
from concourse import bacc
from concourse import bass_isa
from concourse.bass_utils import run_bass_kernel_spmd

F32 = mybir.dt.float32
BF16 = mybir.dt.bfloat16
BF = ml_dtypes.bfloat16

D = 2048          # model dim
SQ = 1024         # new tokens per batch
SC = 1024         # cached tokens
SKV = SC + SQ     # total keys
HD = 128          # head dim
HLOC = 8          # heads per core
DH = HLOC * HD    # per-core projected dim (1024)
KT = 16           # contraction tiles (2048 / 128)
NCORES = 8
P = 128

EXP = mybir.ActivationFunctionType.Exp


def _emit(tc, nc, prm):
    with (
        tc.tile_pool(name="res", bufs=1) as res,
        tc.tile_pool(name="wres", bufs=1) as wres,
        tc.tile_pool(name="xs", bufs=4) as xs,
        tc.tile_pool(name="ptp", bufs=2) as ptp,
        tc.tile_pool(name="trp", bufs=2) as trp,
        tc.tile_pool(name="accp", bufs=2) as accp,
        tc.tile_pool(name="dnp", bufs=2) as dnp,
        tc.tile_pool(name="outs", bufs=4) as outs,
        tc.tile_pool(name="ps8", bufs=1, space="PSUM") as ps8,
    ):
        qt = [res.tile([P, SQ], BF16, name=f"qt{h}", tag=f"qt{h}") for h in range(HLOC)]
        kt = [res.tile([P, SKV], BF16, name=f"kt{h}", tag=f"kt{h}") for h in range(HLOC)]
        vv = [res.tile([P, DH], BF16, name=f"vv{t}", tag=f"vv{t}") for t in range(16)]
        at = [res.tile([P, SQ], BF16, name=f"at{h}", tag=f"at{h}") for h in range(HLOC)]
        tri = res.tile([P, P], BF16, name="tri", tag="tri")
        bq2 = res.tile([P, 8], F32, name="bq2", tag="bq2")
        bk2 = res.tile([P, 8], F32, name="bk2", tag="bk2")

        # prologue on the Pool (SWDGE) queue: constants first (needed by
        # the projection phases), then KV-cache loads that trickle in
        # during projections without occupying the SP queue.
        warm = ps8.tile([P, 512], F32, name="warm", tag="x1")
        nc.tensor.matmul(warm[0:1, 0:1], tri[:, 0:1], tri[:, 0:1], start=True, stop=True)
        # first V-phase weight tile via SWDGE so it lands in parallel with xv
        wv00 = xs.tile([P, 512], BF16, name="wv0_0", tag="xs0")
        nc.gpsimd.dma_start(wv00[:], prm["wv"][0:P, 0:512])

        # ---------------- projections: V, then K, then Q ----------------
        # Resident operands live in four [P, 4, *] batch tiles (one DMA
        # per 4 k-tiles amortizes the per-DMA fixed cost); streams are
        # [P, 4, 512] batches in a 3-deep ring. k-tile k = (g=k//4, t=k%4).
        def bload(tile_, name, g, cols):
            nc.sync.dma_start(
                tile_[:],
                prm[name][512 * g : 512 * (g + 1), cols].rearrange("(t p) n -> p t n", p=P),
            )

        # V: natural layout [tok, feat]; x resident, w streamed.
        xvb = [wres.tile([P, 4, SQ], BF16, name=f"xv{g}", tag=f"big{g}") for g in range(4)]
        wvbs = {}
        for cd in range(2):
            for g in range(4):
                wvbs[(cd, g)] = xs.tile([P, 4, 512], BF16, name=f"wv{cd}_{g}", tag="xs")
        # startup: smallest pieces first so the k=0 matmuls begin ASAP
        nc.sync.dma_start(xvb[0][:, 0, :], prm["xv"][0:P, :])
        nc.scalar.dma_start(
            wvbs[(0, 0)][:, 1:4, :],
            prm["wv"][P:512, 0:512].rearrange("(t p) n -> p t n", p=P),
        )
        nc.sync.dma_start(xvb[0][:, 1, :], prm["xv"][P : 2 * P, :])
        nc.sync.dma_start(
            xvb[0][:, 2:4, :],
            prm["xv"][2 * P : 512, :].rearrange("(t p) n -> p t n", p=P),
        )
        nc.sync.dma_start(
            xvb[1][:, 0:2, :],
            prm["xv"][512:768, :].rearrange("(t p) n -> p t n", p=P),
        )
        nc.sync.dma_start(
            xvb[1][:, 2:4, :],
            prm["xv"][768:1024, :].rearrange("(t p) n -> p t n", p=P),
        )
        bload(wvbs[(0, 1)], "wv", 1, slice(0, 512))
        nc.sync.dma_start(
            xvb[2][:, 0:2, :],
            prm["xv"][1024:1280, :].rearrange("(t p) n -> p t n", p=P),
        )
        nc.sync.dma_start(
            xvb[2][:, 2:4, :],
            prm["xv"][1280:1536, :].rearrange("(t p) n -> p t n", p=P),
        )
        bload(wvbs[(0, 2)], "wv", 2, slice(0, 512))
        bload(xvb[3], "xv", 3, slice(0, SQ))
        bload(wvbs[(0, 3)], "wv", 3, slice(0, 512))
        # cd1 weight stream + constants + KV cache trickle in on the Pool
        # queue; SP is left free for the K-phase prefetch during cd1.
        for g in range(4):
            bload(wvbs[(1, g)], "wv", g, slice(512, 1024))
        nc.gpsimd.dma_start(tri[:], prm["tri"][:])
        nc.gpsimd.dma_start(bq2[:], prm["bq2"][:])
        nc.gpsimd.dma_start(bk2[:], prm["bk2"][:])
        for h in range(HLOC):
            nc.gpsimd.dma_start(kt[h][:, 0:SC], prm["ckt"][P * h : P * (h + 1), :])
        for t in range(8):
            nc.gpsimd.dma_start(vv[t][:], prm["cv"][P * t : P * (t + 1), :])

        def proj_ps(pref):
            d0 = ps8.tile([P, 1024], F32, name=f"{pref}d0", tag="d0")
            d1 = ps8.tile([P, 1024], F32, name=f"{pref}d1", tag="d1")
            return [
                d0[:, 0:512], d0[:, 512:1024], d1[:, 0:512], d1[:, 512:1024],
                ps8.tile([P, 512], F32, name=f"{pref}o0", tag="o0")[:],
                ps8.tile([P, 512], F32, name=f"{pref}o1", tag="o1")[:],
                ps8.tile([P, 512], F32, name=f"{pref}x0", tag="x0")[:],
                ps8.tile([P, 512], F32, name=f"{pref}x1", tag="x1")[:],
            ]

        for cd in range(2):
            ps = proj_ps(f"vps{cd}_")

            for g in range(4):
                wvb = wvbs[(cd, g)]
                for t in range(4):
                    k = 4 * g + t
                    rhs = wv00[:] if (cd == 0 and k == 0) else wvb[:, t, :]
                    for m in range(8):
                        nc.tensor.matmul(
                            ps[m], xvb[g][:, t, P * m : P * (m + 1)], rhs,
                            start=(k == 0), stop=(k == KT - 1),
                        )
                        if k == KT - 1:
                            # drain the PSUM->SBUF copy while later m's finish
                            dsl = vv[8 + m][:, 512 * cd : 512 * (cd + 1)]
                            if m % 2 == 0:
                                nc.scalar.copy(dsl, ps[m])
                            else:
                                nc.vector.tensor_copy(dsl, ps[m])

        # K then Q: transposed layout [feat, tok]; w resident, x streamed.
        # Q chunk 1 is deferred into the attention-c0 window (ACT-bound
        # there, so its matmuls fill otherwise-idle PE time).
        wtq = None
        for wname, xname, dest, col0, bias2, cs in (
            ("wk", "xk", kt, SC, bk2, (0, 1)),
            ("wq", "xq", qt, 0, bq2, (0,)),
        ):
            wt = [wres.tile([P, 4, DH], BF16, name=f"{wname}{g}", tag=f"big{g}") for g in range(4)]
            if wname == "wq":
                wtq = wt
            for c in cs:
                ps = proj_ps(f"{wname}ps{c}_")
                for g in range(4):
                    if c == 0:
                        bload(wt[g], wname, g, slice(0, DH))
                    xb = xs.tile([P, 4, 512], BF16, name=f"{xname}{c}_{g}", tag="xs")
                    bload(xb, xname, g, slice(512 * c, 512 * (c + 1)))
                    for t in range(4):
                        k = 4 * g + t
                        for m in range(8):
                            nc.tensor.matmul(
                                ps[m], wt[g][:, t, P * m : P * (m + 1)], xb[:, t, :],
                                start=(k == 0), stop=(k == KT - 1),
                            )
                            if k == KT - 1:
                                dsl = dest[m][:, col0 + 512 * c : col0 + 512 * c + 512]
                                if m % 2 == 0:
                                    nc.scalar.add(dsl, ps[m], bias2[:, m : m + 1])
                                else:
                                    nc.vector.tensor_scalar_add(
                                        dsl, ps[m], bias2[:, m : m + 1]
                                    )

        # ---------------- attention ----------------
        def qc1_pair(p):
            # deferred Q projection for chunk 1, two feature tiles at a time
            # (PSUM banks x0/x1 are free until out-projection starts)
            ps2 = [
                ps8.tile([P, 512], F32, name=f"qps1_{p}_{i}", tag=f"x{i}")
                for i in range(2)
            ]
            for g in range(4):
                xb = xs.tile([P, 4, 512], BF16, name=f"xq1_{p}_{g}", tag="xs")
                bload(xb, "xq", g, slice(512, 1024))
                for t in range(4):
                    k = 4 * g + t
                    for i in range(2):
                        m = 2 * p + i
                        nc.tensor.matmul(
                            ps2[i][:], wtq[g][:, t, P * m : P * (m + 1)], xb[:, t, :],
                            start=(k == 0), stop=(k == KT - 1),
                        )
                        if k == KT - 1:
                            if i == 0:
                                nc.scalar.add(
                                    qt[m][:, 512:1024], ps2[i][:], bq2[:, m : m + 1]
                                )
                            else:
                                nc.vector.tensor_scalar_add(
                                    qt[m][:, 512:1024], ps2[i][:], bq2[:, m : m + 1]
                                )


        wob = [wres.tile([P, 4, DH], BF16, name=f"wo{i}", tag=f"big{i}") for i in range(4)]

        def wo_prefetch():
            # wob[2*jh + dh][:, tt, :] = Wo^T rows 512*jh + 128*tt, cols DH*dh ..
            for jh in range(2):
                for dh in range(2):
                    nc.sync.dma_start(
                        wob[2 * jh + dh][:],
                        prm["wo"][
                            512 * jh : 512 * (jh + 1), DH * dh : DH * (dh + 1)
                        ].rearrange("(t p) n -> p t n", p=P),
                    )

        pending = []  # deferred (o_ps, d_all, h, c) normalizes
        pvq = []  # cross-chunk PV pipeline (emitted PDP pairs behind scores)
        PDP = 1

        def flush_normalize():
            o_ps, d_all, h, c = pending.pop(0)
            rec = dnp.tile([P, 512], F32, name=f"rec{h}_{c}", tag="rec")
            nc.vector.reciprocal(rec[:], d_all[:])
            nc.vector.tensor_mul(at[h][:, 512 * c : 512 * (c + 1)], o_ps[:], rec[:])

        def attn(h, c, idx):
            n_full = 8 + 4 * c
            n_kv = n_full + 4
            q_sl = slice(512 * c, 512 * (c + 1))
            ptb = ptp.tile([P, 16, 512], BF16, name=f"ptb{h}_{c}", tag="ptb")
            ts = trp.tile([P, 6, 512], BF16, name=f"ts{h}_{c}", tag="ts")
            acc = accp.tile([P, 512], BF16, name=f"acc{h}_{c}", tag="acc")
            d_all = dnp.tile([P, 512], F32, name=f"d{h}_{c}", tag="d")
            o_ps = ps8.tile([P, 512], F32, name=f"o{h}_{c}", tag=f"o{idx % 2}")

            # bf16 pairwise-tree denominator over the full (non-diagonal)
            # tiles; all operands 2-byte so DVE runs in 2x mode. Pair adds
            # are emitted as exps complete; the tree root lands in acc and
            # the diagonal tiles are then added in place, each starting at
            # its own query block so causality is preserved.
            def tree_root():
                if n_full == 8:
                    nc.vector.tensor_add(ts[:, 4, :], ts[:, 0, :], ts[:, 1, :])
                    nc.vector.tensor_add(ts[:, 5, :], ts[:, 2, :], ts[:, 3, :])
                    nc.vector.tensor_add(acc[:], ts[:, 4, :], ts[:, 5, :])
                else:  # 12 full tiles -> 6 pairs
                    nc.vector.tensor_add(ts[:, 0, :], ts[:, 0, :], ts[:, 1, :])
                    nc.vector.tensor_add(ts[:, 2, :], ts[:, 2, :], ts[:, 3, :])
                    nc.vector.tensor_add(ts[:, 4, :], ts[:, 4, :], ts[:, 5, :])
                    nc.vector.tensor_add(ts[:, 0, :], ts[:, 0, :], ts[:, 2, :])
                    nc.vector.tensor_add(acc[:], ts[:, 0, :], ts[:, 4, :])

            def scores_pair(u):
                # two score tiles per double-bank PSUM tile; full pairs get
                # one [P,1024] exp straight into adjacent ptb slots
                dp = ps8.tile([P, 1024], F32, name=f"st{h}_{c}_{u}", tag=f"d{u % 2}")
                for half in range(2):
                    g = 2 * u + half
                    j = g - n_full
                    base = 512 * half
                    if j < 0:
                        nc.tensor.matmul(
                            dp[:, base : base + 512],
                            kt[h][:, P * g : P * (g + 1)], qt[h][:, q_sl],
                            start=True, stop=True,
                        )
                    else:
                        o0 = P * j
                        nc.tensor.matmul(
                            dp[:, base + o0 : base + 512],
                            kt[h][:, P * g : P * (g + 1)],
                            qt[h][:, 512 * c + o0 : 512 * (c + 1)],
                            start=True, stop=True,
                        )
                g = 2 * u + 1
                if g < n_full:
                    nc.scalar.activation(ptb[:, 2 * u : 2 * u + 2, :], dp[:], EXP)
                    nc.vector.tensor_add(
                        ts[:, u, :], ptb[:, g - 1, :], ptb[:, g, :]
                    )
                    if g == n_full - 1:
                        tree_root()
                else:
                    first_diag = 2 * u == n_full
                    if first_diag:
                        # j=0 spans the full half; one exp covers both tiles
                        # (the 128 garbage cols between are never read)
                        nc.scalar.activation(ptb[:, 2 * u : 2 * u + 2, :], dp[:], EXP)
                    for half in range(2):
                        g = 2 * u + half
                        o0 = P * (g - n_full)
                        base = 512 * half
                        if not first_diag:
                            nc.scalar.activation(
                                ptb[:, g, o0:512], dp[:, base + o0 : base + 512], EXP
                            )
                        nc.vector.tensor_mul(
                            ptb[:, g, o0 : o0 + P], ptb[:, g, o0 : o0 + P], tri[:]
                        )
                        nc.vector.tensor_add(
                            acc[:, o0:512], acc[:, o0:512], ptb[:, g, o0:512]
                        )

            def pv(g):
                j = g - n_full
                if j < 0:
                    nc.tensor.matmul(
                        o_ps[:], vv[g][:, P * h : P * (h + 1)], ptb[:, g, :],
                        start=(g == 0), stop=False,
                    )
                else:
                    o0 = P * j
                    # retiring query block: last writer of cols [o0, o0+128)
                    nc.tensor.matmul(
                        o_ps[:, o0 : o0 + P], vv[g][:, P * h : P * (h + 1)],
                        ptb[:, g, o0 : o0 + P], start=False, stop=True,
                    )
                    if o0 + P < 512:
                        nc.tensor.matmul(
                            o_ps[:, o0 + P : 512], vv[g][:, P * h : P * (h + 1)],
                            ptb[:, g, o0 + P : 512], start=False, stop=(j == 3),
                        )

            def finish():
                nc.gpsimd.partition_all_reduce(
                    d_all[:], acc[:], channels=P, reduce_op=bass_isa.ReduceOp.add
                )
                pending.append((o_ps, d_all, h, c))
                if len(pending) >= 2:
                    flush_normalize()

            # feed the global cross-chunk pipeline: one entry per pair;
            # the last pair carries the chunk finisher (pall + normalize)
            n_pairs = n_kv // 2
            for u in range(n_pairs):
                scores_pair(u)

                def mk(g0=2 * u, fin=(finish if u == n_pairs - 1 else None)):
                    def emit():
                        pv(g0)
                        pv(g0 + 1)
                        if fin is not None:
                            fin()
                    return emit

                pvq.append(mk())
                while len(pvq) > PDP:
                    pvq.pop(0)()

        def outproj(c, ms=range(16), split_last=False):
            for m in ms:
                op = ps8.tile([P, 512], F32, name=f"op{m}_{c}", tag=f"x{m % 2}")
                for t in range(HLOC):
                    nc.tensor.matmul(
                        op[:],
                        wob[2 * (t // 4) + m // 8][:, t % 4, P * (m % 8) : P * (m % 8 + 1)],
                        at[t][:, 512 * c : 512 * (c + 1)],
                        start=(t == 0), stop=(t == HLOC - 1),
                    )
                ob = outs.tile([P, 512], BF16, name=f"ob{m}_{c}", tag="ob")
                if split_last and m == 15:
                    # two engines copy halves in parallel to shorten the tail
                    nc.scalar.copy(ob[:, 0:256], op[:, 0:256])
                    nc.vector.tensor_copy(ob[:, 256:512], op[:, 256:512])
                    nc.sync.dma_start(
                        prm["outT"][P * m : P * (m + 1), 512 * c : 512 * c + 256],
                        ob[:, 0:256],
                    )
                    nc.sync.dma_start(
                        prm["outT"][P * m : P * (m + 1), 512 * c + 256 : 512 * (c + 1)],
                        ob[:, 256:512],
                    )
                else:
                    nc.scalar.copy(ob[:], op[:])
                    nc.sync.dma_start(
                        prm["outT"][P * m : P * (m + 1), 512 * c : 512 * (c + 1)], ob[:]
                    )

        idx = 0
        for h in range(HLOC):
            attn(h, 0, idx)
            idx += 1
            if h in (0, 2, 4):
                qc1_pair(h // 2)
            if h == 6:
                wo_prefetch()
        for h in range(HLOC):
            attn(h, 1, idx)
            idx += 1
            if h == 0:
                qc1_pair(3)
        while pvq:
            pvq.pop(0)()
        while pending:
            flush_normalize()
        outproj(0)
        outproj(1)


def build():
    nc = bacc.Bacc(None, target_bir_lowering=False)
    prm = {}
    for n, shape, dt in (
        ("xq", [D, SQ], BF16),
        ("xk", [D, SQ], BF16),
        ("xv", [D, SQ], BF16),
        ("wq", [D, DH], BF16),
        ("wk", [D, DH], BF16),
        ("wv", [D, DH], BF16),
        ("wo", [DH, D], BF16),
        ("ckt", [DH, SC], BF16),
        ("cv", [SC, DH], BF16),
        ("tri", [P, P], BF16),
        ("bq2", [P, 8], F32),
        ("bk2", [P, 8], F32),
    ):
        prm[n] = nc.declare_dram_parameter(n, shape, dt, isOutput=False)
    prm["outT"] = nc.declare_dram_parameter("outT", [D, SQ], BF16, isOutput=True)
    with tile.TileContext(nc) as tc:
        _emit(tc, nc, prm)
    nc.compile()
    return nc


def make_in_maps(query, key, value, cached_k, cached_v, Wq, bq, Wk, bk, Wv, bv, Wo, bo):
    """Per-core host prep: slice + transpose + casts (bf16 streams)."""
    s = float(np.sqrt(HD))
    tri = np.triu(np.ones((P, P), dtype=np.float32)).astype(BF)

    in_maps = []
    for c in range(NCORES):
        b, h2 = c // 2, c % 2
        hs = slice(DH * h2, DH * (h2 + 1))
        in_maps.append(
            {
                "xq": np.ascontiguousarray(query[b].T).astype(BF),
                "xk": np.ascontiguousarray(key[b].T).astype(BF),
                "xv": np.ascontiguousarray(value[b].T).astype(BF),
                "wq": np.ascontiguousarray(Wq[hs].T / s).astype(BF),
                "wk": np.ascontiguousarray(Wk[hs].T).astype(BF),
                "wv": np.ascontiguousarray(Wv[hs].T).astype(BF),
                "wo": np.ascontiguousarray(Wo[:, hs].T).astype(BF),
                "ckt": np.ascontiguousarray(cached_k[b][:, hs].T).astype(BF),
                "cv": np.ascontiguousarray(cached_v[b][:, hs] - bv[hs]).astype(BF),
                "tri": tri,
                "bq2": np.ascontiguousarray((bq[hs] / s).reshape(8, P).T.astype(np.float32)),
                "bk2": np.ascontiguousarray(bk[hs].reshape(8, P).T.astype(np.float32)),
            }
        )
    return in_maps


_NC_CACHE = []


def get_nc():
    if not _NC_CACHE:
        _NC_CACHE.append(build())
    return _NC_CACHE[0]


def assemble(results, bo):
    out = np.empty((4, SQ, D), dtype=np.float32)
    for b in range(4):
        acc = results[2 * b]["outT"].astype(np.float32) + results[
            2 * b + 1
        ]["outT"].astype(np.float32)  # [D, SQ]
        out[b] = acc.T + bo[None, :]
    return out


def kernel(query, key, value, cached_k, cached_v, Wq, bq, Wk, bk, Wv, bv, Wo, bo):
    query = np.asarray(query, dtype=np.float32)
    key = np.asarray(key, dtype=np.float32)
    value = np.asarray(value, dtype=np.float32)
    cached_k = np.asarray(cached_k, dtype=np.float32)
    cached_v = np.asarray(cached_v, dtype=np.float32)
    Wq, bq = np.asarray(Wq, np.float32), np.asarray(bq, np.float32)
    Wk, bk = np.asarray(Wk, np.float32), np.asarray(bk, np.float32)
    Wv, bv = np.asarray(Wv, np.float32), np.asarray(bv, np.float32)
    Wo, bo = np.asarray(Wo, np.float32), np.asarray(bo, np.float32)

    nc = get_nc()
    in_maps = make_in_maps(
        query, key, value, cached_k, cached_v, Wq, bq, Wk, bk, Wv, bv, Wo, bo
    )
    res = run_bass_kernel_spmd(nc, in_maps, list(range(NCORES)))
    return assemble(res.results, bo + bv @ Wo.T)
